# revision 55
# baseline (speedup 1.0000x reference)
"""
DistancePredictor Trainium2 kernel.

Math:
  xi = x @ Wi + bi            [B, L, H]
  xj = x @ Wj + bj            [B, L, H]
  out = relu(xi[:,:,None,:] * xj[:,None,:,:]) @ Wo + bo    [B, L, L, NB]

Key identity (exact, terms have disjoint support):
  relu(a*b) = relu(a)relu(b) + relu(-a)relu(-b)
so
  out[i,j,n] = sum_h (A+[i,h]B+[j,h] + A-[i,h]B-[j,h]) * Wo[h,n] + bo[n]
with A± = relu(±xi), B± = relu(±xj) — the whole pair/relu/contract
pipeline is pure TensorE matmuls; no [B,L,L,H] intermediate exists.

Signs as implemented:
  atp = max(psA,0) *  Wo      (psA = xi+bi, bias folded in via rank-1 matmul)
  atm = min(psA,0) * (-Wo)    (= relu(-(xi+bi)) * Wo)   [t0, fused from PSUM]
  atm = relu(-psA) *  Wo                                 [t1, via SBUF copy]
  bp  = max(psB,0)            (psB = xj+bj)
  bm  = max(-psB,0)           (= relu(-(xj+bj)))
  out[n] = atp·bp + atm·bm + bo[n]

Sharding: 8 cores; core c handles batch b=c//4 and i-rows
[96*(c%4), 96*(c%4)+96).  Weights replicated.

Schedule:
 - Inputs stream in packed layouts (1.5-2.5KB contiguous per partition
   per DMA) over both HW rings + one wi group via the gpsimd SWDGE
   path; the PE consumes chunks in arrival order (A data early, wj
   spread) so layer 1 finishes with the stream, and psA closes ~1us
   before the last wj so the at±-chain hides under the B tail.
 - Biases enter the PSUM accumulations as rank-1 matmuls (ones ⊗ b):
   post-accumulation ops are single fused DVE/ACT ops.
 - at±-chain split: Vector owns t0 (fused PSUM reads), GpSimd owns t1
   (from SBUF relu copies), Scalar owns bp + output biases.
 - Junk matmuls on a memset tile keep TensorE gapless so the HAM clock
   ramps to full (~5us of continuous activity) before the main loop.
 - Output staged per-n in fp16 (~5e-4 added rel err), drained on the
   sync ring during the main loop; the last n goes out on the scalar
   ring right behind its bias ACT.
"""

import numpy as np
import ml_dtypes

import concourse.bass as bass
import concourse.mybir as mybir
import concourse.tile as tile
from concourse import bacc, bass_utils

# Problem constants (hardcoded per contract).
B, L, D, H, NB = 2, 384, 1280, 256, 10
P = 128
KT = D // P     # 10 contraction chunks of 128
HT = H // P     # 2 h-chunks of 128
NCORES = 8
IB = (B * L) // NCORES   # 96 i-rows per core

F32 = mybir.dt.float32
F16 = mybir.dt.float16
ALU = mybir.AluOpType
RELU = mybir.ActivationFunctionType.Relu
IDENT = mybir.ActivationFunctionType.Identity

_last_result = None  # BassKernelResults of the most recent run (for test harness)


def build_nc():
    nc = bacc.Bacc("TRN2")

    xbt = nc.dram_tensor("xbt", [5, P, 2, L], F16, kind="ExternalInput")
    wi0 = nc.dram_tensor("wi0", [P, 5, H], F16, kind="ExternalInput")   # k0-4
    wi1 = nc.dram_tensor("wi1", [P, 5, H], F16, kind="ExternalInput")   # k5-9
    wja = nc.dram_tensor("wja", [P, 4, H], F16, kind="ExternalInput")   # k0-3
    wjb = nc.dram_tensor("wjb", [P, 4, H], F16, kind="ExternalInput")   # k4-7
    wjc = nc.dram_tensor("wjc", [P, 2, H], F16, kind="ExternalInput")   # k8-9
    # cst[:, 0:2] = Wo per h-chunk, [:, 2:4] = -Wo, [:, 4] = bo replicated
    cst = nc.dram_tensor("cst", [P, 5, NB], F32, kind="ExternalInput")
    # bias rows on one partition: [bi_t0, bi_t1, bj_t0, bj_t1]
    brow = nc.dram_tensor("brow", [1, 4, P], F16, kind="ExternalInput")
    out = nc.dram_tensor("out", [NB, IB, L], F16, kind="ExternalOutput")

    with tile.TileContext(nc) as tc:
        with (
            tc.tile_pool(name="persist", bufs=1) as pp,
            tc.tile_pool(name="psA", bufs=2, space="PSUM") as psA_pool,
            tc.tile_pool(name="psB", bufs=2, space="PSUM") as psB_pool,
            tc.tile_pool(name="psO", bufs=4, space="PSUM") as psO_pool,
            tc.tile_pool(name="stage", bufs=4) as stage_pool,
        ):
            tl = lambda shape, name, dt=F32: pp.tile(shape, dt, name=name, tag=name)
            xbt_sb = tl([P, KT, L], "xbt_sb", F16)
            wi_sb = tl([P, KT, H], "wi_sb", F16)
            wj_sb = tl([P, KT, H], "wj_sb", F16)
            cst_sb = tl([P, 5, NB], "cst_sb")
            brow_sb = tl([1, 4, P], "brow_sb", F16)
            ones_sb = tl([1, L], "ones_sb", F16)

            bp_sb = tl([P, HT, L], "bp_sb", F16)         # relu(xj+bj)      [h, j]
            bm_sb = tl([P, HT, L], "bm_sb", F16)         # relu(-(xj+bj))
            atp_sb = tl([P, HT, NB, IB], "atp_sb", F16)  # [h, n, i]
            atm_sb = tl([P, HT, NB, IB], "atm_sb", F16)
            ap1_sb = tl([P, IB], "ap1_sb")               # max(psA1, 0)
            am1_sb = tl([P, IB], "am1_sb")               # relu(-psA1)

            warm_sb = tl([P, L], "warm_sb", F16)
            nc.vector.memset(warm_sb[:], 0.0)
            nc.vector.memset(ones_sb[:], 1.0)

            def junk(n_junk):
                # Full 128-partition matmuls: the HAM clock monitor only
                # counts wide-K PE streaming (K=32/64 never ramps).
                psW = psO_pool.tile([IB, L], F32, name="psW", tag="psO")
                for _ in range(n_junk):
                    nc.tensor.matmul(psW[:], warm_sb[:, :IB], warm_sb[:],
                                     start=True, stop=True,
                                     skip_group_check=True)

            # ---- DMA triggers.  Emission order per engine = issue order.
            # The rings share ~330-350GB/s of HBM; A-side data (xbt+wi)
            # is front-loaded on both rings so psA closes ~1us before the
            # stream ends, with wj groups spread in between so B matmuls
            # never bunch.  wjc is consumed last, h-split so psB[0]
            # closes before psB[1].
            xbt_v = xbt_sb[:].rearrange("p (g k) j -> p g k j", k=2)
            # jc rides the (otherwise idle) SWDGE path: ~130GB/s while the
            # HW rings are in their slow-start phase; takes 131KB off the
            # rings and lands the b(8,9) data by ~10.5us.
            nc.gpsimd.dma_start(wj_sb[:, 8:10, :], wjc[:])
            nc.gpsimd.dma_start(wj_sb[:, 2:4, :], wja[:, 2:4])
            nc.sync.dma_start(cst_sb[:], cst[:])
            nc.scalar.dma_start(brow_sb[:], brow[:])
            # sync: xbt0, xbt1, ja0(k0,1), xbt2, xbt3, jb0(k4,5), jc(k8,9)
            nc.sync.dma_start(xbt_v[:, 0], xbt[0])
            nc.sync.dma_start(xbt_v[:, 1], xbt[1])
            nc.sync.dma_start(wj_sb[:, 0:2, :], wja[:, 0:2])
            nc.sync.dma_start(xbt_v[:, 2], xbt[2])
            nc.sync.dma_start(xbt_v[:, 3], xbt[3])
            nc.sync.dma_start(wj_sb[:, 4:6, :], wjb[:, 0:2])
            # scalar: wi0, wi1, xbt4, jb1(k6,7)
            nc.scalar.dma_start(wi_sb[:, 0:5, :], wi0[:])
            nc.scalar.dma_start(wi_sb[:, 5:10, :], wi1[:])
            nc.scalar.dma_start(xbt_v[:, 4], xbt[4])
            nc.scalar.dma_start(wj_sb[:, 6:8, :], wjb[:, 2:4])

            psA = [psA_pool.tile([P, IB], F32, name="psA", tag="psA")
                   for _ in range(HT)]
            psB = [psB_pool.tile([P, L], F32, name="psB", tag="psB")
                   for _ in range(HT)]

            junk(3)

            # ---- bias rank-1 matmuls open each accumulation group ----
            for t in range(HT):
                nc.tensor.matmul(psA[t][:], brow_sb[:, t], ones_sb[:, :IB],
                                 start=True, stop=False)
            for t in range(HT):
                nc.tensor.matmul(psB[t][:], brow_sb[:, 2 + t], ones_sb[:],
                                 start=True, stop=False)

            # ---- layer 1: consume chunks in expected arrival order ----
            def a_chunks(ks, sp=False):
                for k in ks:
                    for t in range(HT):
                        nc.tensor.matmul(psA[t][:],
                                         wi_sb[:, k, t * P:(t + 1) * P],
                                         xbt_sb[:, k, :IB],
                                         start=False,
                                         stop=sp and k == ks[-1] and t == HT - 1)

            def b_chunks(ks, sp=False):
                for k in ks:
                    for t in range(HT):
                        nc.tensor.matmul(psB[t][:],
                                         wj_sb[:, k, t * P:(t + 1) * P],
                                         xbt_sb[:, k, :],
                                         start=False,
                                         stop=sp and k == ks[-1] and t == HT - 1)

            junk(4)
            a_chunks([0, 1])
            junk(2)
            a_chunks([2, 3])
            junk(2)
            b_chunks([0, 1])
            junk(1)
            a_chunks([4])
            b_chunks([2, 3])
            junk(1)
            a_chunks([5])
            a_chunks([6, 7])
            junk(1)
            a_chunks([8, 9], sp=True)
            junk(1)
            b_chunks([8, 9])
            junk(1)
            b_chunks([4, 5])
            # b(6,7) h-consume-split tail: t0 matmuls first so psB[0]
            # closes early and its relu ACTs overlap the t1 matmuls.
            for t in range(HT):
                for k in (6, 7):
                    nc.tensor.matmul(psB[t][:],
                                     wj_sb[:, k, t * P:(t + 1) * P],
                                     xbt_sb[:, k, :],
                                     start=False, stop=k == 7)
            junk(2)

            # ---- fused post-ops ----
            wo_b = lambda s, lo, hi: cst_sb[:, s, lo:hi, None].to_broadcast(
                (P, hi - lo, IB))
            psa_b = lambda lo, hi: psA[0][:, None, :].to_broadcast(
                (P, hi - lo, IB))

            def at0_op(sign, lo, hi):
                # fused t0: atp = max(psA,0)*Wo ; atm = min(psA,0)*(-Wo)
                dst = (atp_sb if sign == 0 else atm_sb)[:, 0, lo:hi]
                op0 = ALU.max if sign == 0 else ALU.min
                nc.vector.scalar_tensor_tensor(dst, psa_b(lo, hi), 0.0,
                                               wo_b(2 * sign, lo, hi),
                                               op0, ALU.mult)

            def at1_op(sign, lo, hi):
                # t1 on gpsimd from SBUF; both a-parts non-negative -> +Wo.
                src = ap1_sb if sign == 0 else am1_sb
                dst = (atp_sb if sign == 0 else atm_sb)[:, 1, lo:hi]
                nc.gpsimd.tensor_tensor(
                    dst, src[:, None, :].to_broadcast((P, hi - lo, IB)),
                    wo_b(1, lo, hi), ALU.mult)

            # vector: ap1 copy, fused t0 chain, then bm relus
            # b-relus split in j-halves: the first halves land ~0.3us after
            # each psB closes, so n0's (j-split) matmuls start that much
            # sooner.
            HLF = L // 2
            nc.vector.tensor_scalar_max(ap1_sb[:], psA[1][:], 0.0)
            at0_op(1, 0, 2)
            at0_op(0, 0, 2)
            for t in range(HT):
                for h0 in (0, HLF):
                    nc.vector.tensor_scalar(bm_sb[:, t, h0:h0 + HLF],
                                            psB[t][:, h0:h0 + HLF], -1.0, 0.0,
                                            ALU.mult, ALU.max)
            at0_op(1, 2, 6)
            at0_op(0, 2, 6)
            at0_op(1, 6, 10)
            at0_op(0, 6, 10)
            # scalar: am1 copy, bp relus (psB[0] closes first)
            nc.scalar.activation(am1_sb[:], psA[1][:], RELU, scale=-1.0)
            for t in range(HT):
                for h0 in (0, HLF):
                    nc.scalar.activation(bp_sb[:, t, h0:h0 + HLF],
                                         psB[t][:, h0:h0 + HLF], RELU)
            # gpsimd: t1 chain
            at1_op(0, 0, 2)
            at1_op(1, 0, 2)
            at1_op(0, 2, 6)
            at1_op(1, 2, 6)
            at1_op(0, 6, 10)
            at1_op(1, 6, 10)

            # ---- main contraction ----
            def main_mm(ps, n, t, sign, st=False, sp=False):
                at = (atp_sb if sign == 0 else atm_sb)[:, t, n, :]
                b = (bp_sb if sign == 0 else bm_sb)[:, t, :]
                nc.tensor.matmul(ps[:], at, b, start=st, stop=sp)

            def bias_out(n, psO, half=False):
                ostage = stage_pool.tile([IB, L], F16, name="ostage", tag="ostage")
                bo_ap = cst_sb[:IB, 4, n:n + 1]
                if half:
                    # split the last n so its first half streams out while
                    # the second half is still converting
                    h = L // 2
                    nc.scalar.activation(ostage[:, :h], psO[:, :h], IDENT,
                                         bias=bo_ap, scale=1.0)
                    nc.scalar.dma_start(out[n, :, :h], ostage[:, :h])
                    nc.scalar.activation(ostage[:, h:], psO[:, h:], IDENT,
                                         bias=bo_ap, scale=1.0)
                    nc.scalar.dma_start(out[n, :, h:], ostage[:, h:])
                elif n % 2 == 0:
                    # vector is free once the at-chain drains
                    nc.vector.tensor_scalar_add(ostage[:], psO[:], bo_ap)
                    nc.sync.dma_start(out[n], ostage[:])
                else:
                    nc.scalar.activation(ostage[:], psO[:], IDENT,
                                         bias=bo_ap, scale=1.0)
                    nc.scalar.dma_start(out[n], ostage[:])

            # n0 runs j-half-split so each matmul half is gated only on its
            # own act half (the acts stream in j-halves too).
            psO0 = psO_pool.tile([IB, L], F32, name="psO", tag="psO")
            for h0 in (0, HLF):
                js = slice(h0, h0 + HLF)
                for i, (t, sign) in enumerate(((0, 0), (0, 1), (1, 0), (1, 1))):
                    at = (atp_sb if sign == 0 else atm_sb)[:, t, 0, :]
                    b = (bp_sb if sign == 0 else bm_sb)[:, t, js]
                    nc.tensor.matmul(psO0[:, js], at, b,
                                     start=i == 0, stop=i == 3)
            bias_out(0, psO0)
            for n in range(1, NB):
                psO = psO_pool.tile([IB, L], F32, name="psO", tag="psO")
                main_mm(psO, n, 0, 0, st=True)
                main_mm(psO, n, 0, 1)
                main_mm(psO, n, 1, 0)
                main_mm(psO, n, 1, 1, sp=True)
                bias_out(n, psO, half=(n == NB - 1))

    return nc


def _prep_inputs(x, Wi, bi, Wj, bj, Wo, bo):
    """Build the 8 per-core input maps."""
    f = lambda a: np.ascontiguousarray(np.asarray(a, dtype=np.float32))
    x, Wi, bi, Wj, bj, Wo, bo = map(f, (x, Wi, bi, Wj, bj, Wo, bo))

    # [1280, H] -> per-partition-contiguous [P, k-range, H] blocks
    def wpack(w, k0, k1):
        v = w.astype(np.float16).reshape(KT, P, H)[k0:k1]      # [k, P, H]
        return np.ascontiguousarray(v.transpose(1, 0, 2))      # [P, k, H]

    wi0_p, wi1_p = wpack(Wi, 0, 5), wpack(Wi, 5, 10)
    wja_p, wjb_p, wjc_p = wpack(Wj, 0, 4), wpack(Wj, 4, 8), wpack(Wj, 8, 10)

    wo_r = Wo.reshape(HT, P, NB).transpose(1, 0, 2)            # [128, 2, 10]
    cst = np.ascontiguousarray(np.stack(
        [wo_r[:, 0], wo_r[:, 1], -wo_r[:, 0], -wo_r[:, 1],
         np.tile(bo[None, :], (P, 1))], axis=1)).astype(np.float32)  # [128, 5, 10]
    brow = np.concatenate([bi.reshape(HT, P), bj.reshape(HT, P)],
                          axis=0)[None].astype(np.float16)     # [1, 4, 128]
    brow = np.ascontiguousarray(brow)

    xT = [x[b].T for b in range(B)]                            # [1280, 384]
    in_maps = []
    for c in range(NCORES):
        b, i0 = c // (NCORES // B), (c % (NCORES // B)) * IB
        xc = np.roll(xT[b], -i0, axis=1).astype(np.float16)
        xc = np.ascontiguousarray(xc.reshape(5, 2, P, L).transpose(0, 2, 1, 3))
        in_maps.append({"xbt": xc, "wi0": wi0_p, "wi1": wi1_p,
                        "wja": wja_p, "wjb": wjb_p, "wjc": wjc_p,
                        "cst": cst, "brow": brow})
    return in_maps


def _run(inputs, trace=False):
    global _last_result
    nc = build_nc()
    if not nc.is_finalized():
        nc.finalize()
    in_maps = _prep_inputs(**inputs)
    res = bass_utils.run_bass_kernel_spmd(
        nc, in_maps, core_ids=list(range(NCORES)), trace=trace)
    _last_result = res
    full = np.empty((B, L, L, NB), dtype=np.float32)
    for c in range(NCORES):
        b, i0 = c // (NCORES // B), (c % (NCORES // B)) * IB
        o = res.results[c]["out"].astype(np.float32)   # [NB, IB, L], j rolled
        o = o.transpose(1, 2, 0)                       # -> [i, j_rolled, n]
        full[b, i0:i0 + IB] = np.roll(o, i0, axis=1)
    return full


def kernel(**inputs):
    return _run(inputs, trace=False)


# revision 57
# speedup vs baseline: 1.0447x; 1.0447x over previous
"""
DistancePredictor Trainium2 kernel.

Math:
  xi = x @ Wi + bi            [B, L, H]
  xj = x @ Wj + bj            [B, L, H]
  out = relu(xi[:,:,None,:] * xj[:,None,:,:]) @ Wo + bo    [B, L, L, NB]

Key identity (exact, terms have disjoint support):
  relu(a*b) = relu(a)relu(b) + relu(-a)relu(-b)
so
  out[i,j,n] = sum_h (A+[i,h]B+[j,h] + A-[i,h]B-[j,h]) * Wo[h,n] + bo[n]
with A± = relu(±xi), B± = relu(±xj) — the whole pair/relu/contract
pipeline is pure TensorE matmuls; no [B,L,L,H] intermediate exists.

Signs as implemented:
  atp = max(psA,0) *  Wo      (psA = xi+bi, bias folded in via rank-1 matmul)
  atm = min(psA,0) * (-Wo)    (= relu(-(xi+bi)) * Wo)   [t0, fused from PSUM]
  atm = relu(-psA) *  Wo                                 [t1, via SBUF copy]
  bp  = max(psB,0)            (psB = xj+bj)
  bm  = max(-psB,0)           (= relu(-(xj+bj)))
  out[n] = atp·bp + atm·bm + bo[n]

Sharding: 8 cores; core c handles batch b=c//4 and i-rows
[96*(c%4), 96*(c%4)+96).  Weights replicated.

Schedule:
 - Inputs stream in packed layouts (1.5-2.5KB contiguous per partition
   per DMA) over both HW rings + one wi group via the gpsimd SWDGE
   path; the PE consumes chunks in arrival order (A data early, wj
   spread) so layer 1 finishes with the stream, and psA closes ~1us
   before the last wj so the at±-chain hides under the B tail.
 - Biases enter the PSUM accumulations as rank-1 matmuls (ones ⊗ b):
   post-accumulation ops are single fused DVE/ACT ops.
 - at±-chain split: Vector owns t0 (fused PSUM reads), GpSimd owns t1
   (from SBUF relu copies), Scalar owns bp + output biases.
 - Junk matmuls on a memset tile keep TensorE gapless so the HAM clock
   ramps to full (~5us of continuous activity) before the main loop.
 - Output staged per-n in fp16 (~5e-4 added rel err), drained on the
   sync ring during the main loop; the last n goes out on the scalar
   ring right behind its bias ACT.
"""

import numpy as np
import ml_dtypes

import concourse.bass as bass
import concourse.mybir as mybir
import concourse.tile as tile
from concourse import bacc, bass_utils

# Problem constants (hardcoded per contract).
B, L, D, H, NB = 2, 384, 1280, 256, 10
P = 128
KT = D // P     # 10 contraction chunks of 128
HT = H // P     # 2 h-chunks of 128
NCORES = 8
IB = (B * L) // NCORES   # 96 i-rows per core

F32 = mybir.dt.float32
F16 = mybir.dt.float16
ALU = mybir.AluOpType
RELU = mybir.ActivationFunctionType.Relu
IDENT = mybir.ActivationFunctionType.Identity

_last_result = None  # BassKernelResults of the most recent run (for test harness)


def build_nc():
    nc = bacc.Bacc("TRN2")

    xbt = nc.dram_tensor("xbt", [5, P, 2, L], F16, kind="ExternalInput")
    wi0 = nc.dram_tensor("wi0", [P, 5, H], F16, kind="ExternalInput")   # k0-4
    wi1 = nc.dram_tensor("wi1", [P, 5, H], F16, kind="ExternalInput")   # k5-9
    wja = nc.dram_tensor("wja", [P, 4, H], F16, kind="ExternalInput")   # k0-3
    wjb = nc.dram_tensor("wjb", [P, 4, H], F16, kind="ExternalInput")   # k4-7
    wjc = nc.dram_tensor("wjc", [P, 2, H], F16, kind="ExternalInput")   # k8-9
    # cst[:, 0:2] = Wo per h-chunk, [:, 2:4] = -Wo, [:, 4] = bo replicated
    cst = nc.dram_tensor("cst", [P, 5, NB], F32, kind="ExternalInput")
    # bias rows on one partition: [bi_t0, bi_t1, bj_t0, bj_t1]
    brow = nc.dram_tensor("brow", [1, 4, P], F16, kind="ExternalInput")
    out = nc.dram_tensor("out", [NB, IB, L], F16, kind="ExternalOutput")

    with tile.TileContext(nc) as tc:
        with (
            tc.tile_pool(name="persist", bufs=1) as pp,
            tc.tile_pool(name="psA", bufs=2, space="PSUM") as psA_pool,
            tc.tile_pool(name="psB", bufs=2, space="PSUM") as psB_pool,
            tc.tile_pool(name="psO", bufs=4, space="PSUM") as psO_pool,
            tc.tile_pool(name="stage", bufs=4) as stage_pool,
        ):
            tl = lambda shape, name, dt=F32: pp.tile(shape, dt, name=name, tag=name)
            xbt_sb = tl([P, KT, L], "xbt_sb", F16)
            wi_sb = tl([P, KT, H], "wi_sb", F16)
            wj_sb = tl([P, KT, H], "wj_sb", F16)
            cst_sb = tl([P, 5, NB], "cst_sb")
            brow_sb = tl([1, 4, P], "brow_sb", F16)
            ones_sb = tl([1, L], "ones_sb", F16)

            bp_sb = tl([P, HT, L], "bp_sb", F16)         # relu(xj+bj)      [h, j]
            bm_sb = tl([P, HT, L], "bm_sb", F16)         # relu(-(xj+bj))
            atp_sb = tl([P, HT, NB, IB], "atp_sb", F16)  # [h, n, i]
            atm_sb = tl([P, HT, NB, IB], "atm_sb", F16)
            ap1_sb = tl([P, IB], "ap1_sb")               # max(psA1, 0)
            am1_sb = tl([P, IB], "am1_sb")               # relu(-psA1)

            warm_sb = tl([P, L], "warm_sb", F16)
            nc.vector.memset(warm_sb[:], 0.0)
            nc.vector.memset(ones_sb[:], 1.0)

            def junk(n_junk):
                # Full 128-partition matmuls: the HAM clock monitor only
                # counts wide-K PE streaming (K=32/64 never ramps).
                psW = psO_pool.tile([IB, L], F32, name="psW", tag="psO")
                for _ in range(n_junk):
                    nc.tensor.matmul(psW[:], warm_sb[:, :IB], warm_sb[:],
                                     start=True, stop=True,
                                     skip_group_check=True)

            # ---- DMA triggers.  Emission order per engine = issue order.
            # The rings share ~330-350GB/s of HBM; A-side data (xbt+wi)
            # is front-loaded on both rings so psA closes ~1us before the
            # stream ends, with wj groups spread in between so B matmuls
            # never bunch.  wjc is consumed last, h-split so psB[0]
            # closes before psB[1].
            xbt_v = xbt_sb[:].rearrange("p (g k) j -> p g k j", k=2)
            # jc rides the (otherwise idle) SWDGE path: ~130GB/s while the
            # HW rings are in their slow-start phase; takes 131KB off the
            # rings and lands the b(8,9) data by ~10.5us.
            nc.gpsimd.dma_start(wj_sb[:, 8:10, :], wjc[:])
            nc.gpsimd.dma_start(wj_sb[:, 2:4, :], wja[:, 2:4])
            nc.sync.dma_start(cst_sb[:], cst[:])
            nc.scalar.dma_start(brow_sb[:], brow[:])
            # sync: xbt0, xbt1, ja0(k0,1), xbt2, xbt3, jb0(k4,5), jc(k8,9)
            nc.sync.dma_start(xbt_v[:, 0], xbt[0])
            nc.sync.dma_start(xbt_v[:, 1], xbt[1])
            nc.sync.dma_start(wj_sb[:, 0:2, :], wja[:, 0:2])
            nc.sync.dma_start(xbt_v[:, 2], xbt[2])
            nc.sync.dma_start(xbt_v[:, 3], xbt[3])
            nc.sync.dma_start(wj_sb[:, 4:6, :], wjb[:, 0:2])
            # scalar: wi0, wi1, xbt4, jb1(k6,7)
            nc.scalar.dma_start(wi_sb[:, 0:5, :], wi0[:])
            nc.scalar.dma_start(wi_sb[:, 5:10, :], wi1[:])
            nc.scalar.dma_start(xbt_v[:, 4], xbt[4])
            nc.scalar.dma_start(wj_sb[:, 6:8, :], wjb[:, 2:4])

            psA = [psA_pool.tile([P, IB], F32, name="psA", tag="psA")
                   for _ in range(HT)]
            psB = [psB_pool.tile([P, L], F32, name="psB", tag="psB")
                   for _ in range(HT)]

            junk(3)

            # ---- bias rank-1 matmuls open each accumulation group ----
            for t in range(HT):
                nc.tensor.matmul(psA[t][:], brow_sb[:, t], ones_sb[:, :IB],
                                 start=True, stop=False)
            for t in range(HT):
                nc.tensor.matmul(psB[t][:], brow_sb[:, 2 + t], ones_sb[:],
                                 start=True, stop=False)

            # ---- layer 1: consume chunks in expected arrival order ----
            def a_chunks(ks, sp=False):
                for k in ks:
                    for t in range(HT):
                        nc.tensor.matmul(psA[t][:],
                                         wi_sb[:, k, t * P:(t + 1) * P],
                                         xbt_sb[:, k, :IB],
                                         start=False,
                                         stop=sp and k == ks[-1] and t == HT - 1)

            def b_chunks(ks, sp=False):
                for k in ks:
                    for t in range(HT):
                        nc.tensor.matmul(psB[t][:],
                                         wj_sb[:, k, t * P:(t + 1) * P],
                                         xbt_sb[:, k, :],
                                         start=False,
                                         stop=sp and k == ks[-1] and t == HT - 1)

            junk(4)
            a_chunks([0, 1])
            junk(2)
            a_chunks([2, 3])
            junk(2)
            b_chunks([0, 1])
            junk(1)
            a_chunks([4])
            b_chunks([2, 3])
            junk(1)
            a_chunks([5])
            a_chunks([6, 7])
            junk(1)
            a_chunks([8, 9], sp=True)
            junk(1)
            b_chunks([8, 9])
            junk(1)
            b_chunks([4, 5])
            # b(6,7) h-consume-split tail: t0 matmuls first so psB[0]
            # closes early and its relu ACTs overlap the t1 matmuls.
            for t in range(HT):
                for k in (6, 7):
                    nc.tensor.matmul(psB[t][:],
                                     wj_sb[:, k, t * P:(t + 1) * P],
                                     xbt_sb[:, k, :],
                                     start=False, stop=k == 7)
            junk(2)

            # ---- fused post-ops ----
            wo_b = lambda s, lo, hi: cst_sb[:, s, lo:hi, None].to_broadcast(
                (P, hi - lo, IB))
            psa_b = lambda lo, hi: psA[0][:, None, :].to_broadcast(
                (P, hi - lo, IB))

            def at0_op(sign, lo, hi):
                # fused t0: atp = max(psA,0)*Wo ; atm = min(psA,0)*(-Wo)
                dst = (atp_sb if sign == 0 else atm_sb)[:, 0, lo:hi]
                op0 = ALU.max if sign == 0 else ALU.min
                nc.vector.scalar_tensor_tensor(dst, psa_b(lo, hi), 0.0,
                                               wo_b(2 * sign, lo, hi),
                                               op0, ALU.mult)

            def at1_op(sign, lo, hi):
                # t1 on gpsimd from SBUF; both a-parts non-negative -> +Wo.
                src = ap1_sb if sign == 0 else am1_sb
                dst = (atp_sb if sign == 0 else atm_sb)[:, 1, lo:hi]
                nc.gpsimd.tensor_tensor(
                    dst, src[:, None, :].to_broadcast((P, hi - lo, IB)),
                    wo_b(1, lo, hi), ALU.mult)

            # vector: ap1 copy, fused t0 chain, then bm relus
            nc.vector.tensor_scalar_max(ap1_sb[:], psA[1][:], 0.0)
            at0_op(1, 0, 2)
            at0_op(0, 0, 2)
            nc.vector.tensor_scalar(bm_sb[:, 0], psB[0][:], -1.0, 0.0,
                                    ALU.mult, ALU.max)
            nc.vector.tensor_scalar(bm_sb[:, 1], psB[1][:], -1.0, 0.0,
                                    ALU.mult, ALU.max)
            at0_op(1, 2, 6)
            at0_op(0, 2, 6)
            at0_op(1, 6, 10)
            at0_op(0, 6, 10)
            # scalar: am1 copy, bp relus (psB[0] closes first)
            nc.scalar.activation(am1_sb[:], psA[1][:], RELU, scale=-1.0)
            nc.scalar.activation(bp_sb[:, 0], psB[0][:], RELU)
            nc.scalar.activation(bp_sb[:, 1], psB[1][:], RELU)
            # gpsimd: t1 chain
            at1_op(0, 0, 2)
            at1_op(1, 0, 2)
            at1_op(0, 2, 6)
            at1_op(1, 2, 6)
            at1_op(0, 6, 10)
            at1_op(1, 6, 10)

            # ---- main contraction ----
            def main_mm(ps, n, t, sign, st=False, sp=False):
                at = (atp_sb if sign == 0 else atm_sb)[:, t, n, :]
                b = (bp_sb if sign == 0 else bm_sb)[:, t, :]
                nc.tensor.matmul(ps[:], at, b, start=st, stop=sp)

            def bias_out(n, psO, half=False):
                ostage = stage_pool.tile([IB, L], F16, name="ostage", tag="ostage")
                bo_ap = cst_sb[:IB, 4, n:n + 1]
                if half:
                    # split the last n so its first half streams out while
                    # the second half is still converting
                    h = L // 2
                    nc.scalar.activation(ostage[:, :h], psO[:, :h], IDENT,
                                         bias=bo_ap, scale=1.0)
                    nc.scalar.dma_start(out[n, :, :h], ostage[:, :h])
                    nc.scalar.activation(ostage[:, h:], psO[:, h:], IDENT,
                                         bias=bo_ap, scale=1.0)
                    nc.scalar.dma_start(out[n, :, h:], ostage[:, h:])
                elif n % 2 == 0:
                    # vector is free once the at-chain drains
                    nc.vector.tensor_scalar_add(ostage[:], psO[:], bo_ap)
                    nc.sync.dma_start(out[n], ostage[:])
                else:
                    nc.scalar.activation(ostage[:], psO[:], IDENT,
                                         bias=bo_ap, scale=1.0)
                    nc.scalar.dma_start(out[n], ostage[:])

            # n0/n1 interleaved by h-chunk: their t0 matmuls (whose relu
            # acts finish first) fill the wait for the t1 acts.
            psO0 = psO_pool.tile([IB, L], F32, name="psO", tag="psO")
            psO1 = psO_pool.tile([IB, L], F32, name="psO", tag="psO")
            for n, ps in ((0, psO0), (1, psO1)):
                main_mm(ps, n, 0, 0, st=True)
                main_mm(ps, n, 0, 1)
            for n, ps in ((0, psO0), (1, psO1)):
                main_mm(ps, n, 1, 0)
                main_mm(ps, n, 1, 1, sp=True)
            bias_out(0, psO0)
            bias_out(1, psO1)
            for n in range(2, NB):
                psO = psO_pool.tile([IB, L], F32, name="psO", tag="psO")
                main_mm(psO, n, 0, 0, st=True)
                main_mm(psO, n, 0, 1)
                main_mm(psO, n, 1, 0)
                main_mm(psO, n, 1, 1, sp=True)
                bias_out(n, psO, half=(n == NB - 1))

    return nc


def _prep_inputs(x, Wi, bi, Wj, bj, Wo, bo):
    """Build the 8 per-core input maps."""
    f = lambda a: np.ascontiguousarray(np.asarray(a, dtype=np.float32))
    x, Wi, bi, Wj, bj, Wo, bo = map(f, (x, Wi, bi, Wj, bj, Wo, bo))

    # [1280, H] -> per-partition-contiguous [P, k-range, H] blocks
    def wpack(w, k0, k1):
        v = w.astype(np.float16).reshape(KT, P, H)[k0:k1]      # [k, P, H]
        return np.ascontiguousarray(v.transpose(1, 0, 2))      # [P, k, H]

    wi0_p, wi1_p = wpack(Wi, 0, 5), wpack(Wi, 5, 10)
    wja_p, wjb_p, wjc_p = wpack(Wj, 0, 4), wpack(Wj, 4, 8), wpack(Wj, 8, 10)

    wo_r = Wo.reshape(HT, P, NB).transpose(1, 0, 2)            # [128, 2, 10]
    cst = np.ascontiguousarray(np.stack(
        [wo_r[:, 0], wo_r[:, 1], -wo_r[:, 0], -wo_r[:, 1],
         np.tile(bo[None, :], (P, 1))], axis=1)).astype(np.float32)  # [128, 5, 10]
    brow = np.concatenate([bi.reshape(HT, P), bj.reshape(HT, P)],
                          axis=0)[None].astype(np.float16)     # [1, 4, 128]
    brow = np.ascontiguousarray(brow)

    xT = [x[b].T for b in range(B)]                            # [1280, 384]
    in_maps = []
    for c in range(NCORES):
        b, i0 = c // (NCORES // B), (c % (NCORES // B)) * IB
        xc = np.roll(xT[b], -i0, axis=1).astype(np.float16)
        xc = np.ascontiguousarray(xc.reshape(5, 2, P, L).transpose(0, 2, 1, 3))
        in_maps.append({"xbt": xc, "wi0": wi0_p, "wi1": wi1_p,
                        "wja": wja_p, "wjb": wjb_p, "wjc": wjc_p,
                        "cst": cst, "brow": brow})
    return in_maps


def _run(inputs, trace=False):
    global _last_result
    nc = build_nc()
    if not nc.is_finalized():
        nc.finalize()
    in_maps = _prep_inputs(**inputs)
    res = bass_utils.run_bass_kernel_spmd(
        nc, in_maps, core_ids=list(range(NCORES)), trace=trace)
    _last_result = res
    full = np.empty((B, L, L, NB), dtype=np.float32)
    for c in range(NCORES):
        b, i0 = c // (NCORES // B), (c % (NCORES // B)) * IB
        o = res.results[c]["out"].astype(np.float32)   # [NB, IB, L], j rolled
        o = o.transpose(1, 2, 0)                       # -> [i, j_rolled, n]
        full[b, i0:i0 + IB] = np.roll(o, i0, axis=1)
    return full


def kernel(**inputs):
    return _run(inputs, trace=False)


# revision 65
# speedup vs baseline: 1.1269x; 1.0787x over previous
"""
DistancePredictor Trainium2 kernel.

Math:
  xi = x @ Wi + bi            [B, L, H]
  xj = x @ Wj + bj            [B, L, H]
  out = relu(xi[:,:,None,:] * xj[:,None,:,:]) @ Wo + bo    [B, L, L, NB]

Key identity (exact, terms have disjoint support):
  relu(a*b) = relu(a)relu(b) + relu(-a)relu(-b)
so
  out[i,j,n] = sum_h (A+[i,h]B+[j,h] + A-[i,h]B-[j,h]) * Wo[h,n] + bo[n]
with A± = relu(±xi), B± = relu(±xj) — the whole pair/relu/contract
pipeline is pure TensorE matmuls; no [B,L,L,H] intermediate exists.

Signs as implemented:
  atp = max(psA,0) *  Wo      (psA = xi+bi, bias folded in via rank-1 matmul)
  atm = min(psA,0) * (-Wo)    (= relu(-(xi+bi)) * Wo)   [t0, fused from PSUM]
  atm = relu(-psA) *  Wo                                 [t1, via SBUF copy]
  bp  = max(psB,0)            (psB = xj+bj)
  bm  = max(-psB,0)           (= relu(-(xj+bj)))
  out[n] = atp·bp + atm·bm + bo[n]

Sharding: 8 cores; core c handles batch b=c//4 and i-rows
[96*(c%4), 96*(c%4)+96).  Weights replicated.

Schedule:
 - Inputs stream in packed layouts (1.5-2.5KB contiguous per partition
   per DMA) over both HW rings + one wi group via the gpsimd SWDGE
   path; the PE consumes chunks in arrival order (A data early, wj
   spread) so layer 1 finishes with the stream, and psA closes ~1us
   before the last wj so the at±-chain hides under the B tail.
 - Biases enter the PSUM accumulations as rank-1 matmuls (ones ⊗ b):
   post-accumulation ops are single fused DVE/ACT ops.
 - at±-chain split: Vector owns t0 (fused PSUM reads), GpSimd owns t1
   (from SBUF relu copies), Scalar owns bp + output biases.
 - Junk matmuls on a memset tile keep TensorE gapless so the HAM clock
   ramps to full (~5us of continuous activity) before the main loop.
 - Output staged per-n in fp16 (~5e-4 added rel err), drained on the
   sync ring during the main loop; the last n goes out on the scalar
   ring right behind its bias ACT.
"""

import numpy as np
import ml_dtypes

import concourse.bass as bass
import concourse.mybir as mybir
import concourse.tile as tile
from concourse import bacc, bass_utils

# Problem constants (hardcoded per contract).
B, L, D, H, NB = 2, 384, 1280, 256, 10
P = 128
KT = D // P     # 10 contraction chunks of 128
HT = H // P     # 2 h-chunks of 128
NCORES = 8
IB = (B * L) // NCORES   # 96 i-rows per core

F32 = mybir.dt.float32
F16 = mybir.dt.float16
ALU = mybir.AluOpType
RELU = mybir.ActivationFunctionType.Relu
IDENT = mybir.ActivationFunctionType.Identity

_last_result = None  # BassKernelResults of the most recent run (for test harness)


def build_nc():
    nc = bacc.Bacc("TRN2")

    xbt = nc.dram_tensor("xbt", [5, P, 2, L], F16, kind="ExternalInput")
    wi0 = nc.dram_tensor("wi0", [P, 5, H], F16, kind="ExternalInput")   # k0-4
    wi1 = nc.dram_tensor("wi1", [P, 5, H], F16, kind="ExternalInput")   # k5-9
    wja = nc.dram_tensor("wja", [P, 4, H], F16, kind="ExternalInput")   # k0-3
    wjb = nc.dram_tensor("wjb", [P, 4, H], F16, kind="ExternalInput")   # k4-7
    wjc = nc.dram_tensor("wjc", [P, 2, H], F16, kind="ExternalInput")   # k8-9
    # cst[:, 0:2] = Wo per h-chunk, [:, 2:4] = -Wo, [:, 4] = bo replicated
    cst = nc.dram_tensor("cst", [P, 5, NB], F32, kind="ExternalInput")
    # bias rows on one partition: [bi_t0, bi_t1, bj_t0, bj_t1]
    brow = nc.dram_tensor("brow", [1, 4, P], F16, kind="ExternalInput")
    # bo replicated over i on one partition (rank-1 output-bias matmul)
    bor = nc.dram_tensor("bor", [1, NB, IB], F16, kind="ExternalInput")
    # [n-half, j-block, j, n, i]: output in j-major blocks (M=128 matmuls)
    out = nc.dram_tensor("out", [2, 3, P, NB // 2, IB], F16, kind="ExternalOutput")

    with tile.TileContext(nc) as tc:
        with (
            tc.tile_pool(name="persist", bufs=1) as pp,
            tc.tile_pool(name="psA", bufs=2, space="PSUM") as psA_pool,
            tc.tile_pool(name="psB", bufs=2, space="PSUM") as psB_pool,
            tc.tile_pool(name="psO", bufs=4, space="PSUM") as psO_pool,
            tc.tile_pool(name="stage", bufs=4) as stage_pool,
        ):
            tl = lambda shape, name, dt=F32: pp.tile(shape, dt, name=name, tag=name)
            xbt_sb = tl([P, KT, L], "xbt_sb", F16)
            wi_sb = tl([P, KT, H], "wi_sb", F16)
            wj_sb = tl([P, KT, H], "wj_sb", F16)
            cst_sb = tl([P, 5, NB], "cst_sb")
            brow_sb = tl([1, 4, P], "brow_sb", F16)
            bor_sb = tl([1, NB, IB], "bor_sb", F16)
            ones_sb = tl([1, L], "ones_sb", F16)

            bp_sb = tl([P, HT, L], "bp_sb", F16)         # relu(xj+bj)      [h, j]
            bm_sb = tl([P, HT, L], "bm_sb", F16)         # relu(-(xj+bj))
            atp_sb = tl([P, HT, NB, IB], "atp_sb", F16)  # [h, n, i]
            atm_sb = tl([P, HT, NB, IB], "atm_sb", F16)
            ap1_sb = tl([P, IB], "ap1_sb")               # max(psA1, 0)
            am1_sb = tl([P, IB], "am1_sb")               # relu(-psA1)

            warm_sb = tl([P, L], "warm_sb", F16)
            nc.vector.memset(warm_sb[:], 0.0)
            nc.vector.memset(ones_sb[:], 1.0)

            def junk(n_junk):
                # Full 128-partition matmuls: the HAM clock monitor only
                # counts wide-K PE streaming (K=32/64 never ramps).
                psW = psO_pool.tile([IB, L], F32, name="psW", tag="psO")
                for _ in range(n_junk):
                    nc.tensor.matmul(psW[:], warm_sb[:, :IB], warm_sb[:],
                                     start=True, stop=True,
                                     skip_group_check=True)

            # ---- DMA triggers.  Emission order per engine = issue order.
            # The rings share ~330-350GB/s of HBM; A-side data (xbt+wi)
            # is front-loaded on both rings so psA closes ~1us before the
            # stream ends, with wj groups spread in between so B matmuls
            # never bunch.  wjc is consumed last, h-split so psB[0]
            # closes before psB[1].
            xbt_v = xbt_sb[:].rearrange("p (g k) j -> p g k j", k=2)
            # jc rides the (otherwise idle) SWDGE path: ~130GB/s while the
            # HW rings are in their slow-start phase; takes 131KB off the
            # rings and lands the b(8,9) data by ~10.5us.
            nc.gpsimd.dma_start(wj_sb[:, 8:10, :], wjc[:])
            nc.gpsimd.dma_start(wj_sb[:, 2:4, :], wja[:, 2:4])
            nc.sync.dma_start(cst_sb[:], cst[:])
            nc.scalar.dma_start(brow_sb[:], brow[:])
            nc.scalar.dma_start(bor_sb[:], bor[:])
            # sync: xbt0, xbt1, ja0(k0,1), xbt2, xbt3, jb0(k4,5), jc(k8,9)
            nc.sync.dma_start(xbt_v[:, 0], xbt[0])
            nc.sync.dma_start(xbt_v[:, 1], xbt[1])
            nc.sync.dma_start(wj_sb[:, 0:2, :], wja[:, 0:2])
            nc.sync.dma_start(xbt_v[:, 2], xbt[2])
            nc.sync.dma_start(xbt_v[:, 3], xbt[3])
            nc.sync.dma_start(wj_sb[:, 4:6, :], wjb[:, 0:2])
            # scalar: wi0, wi1, xbt4, jb1(k6,7)
            nc.scalar.dma_start(wi_sb[:, 0:5, :], wi0[:])
            nc.scalar.dma_start(wi_sb[:, 5:10, :], wi1[:])
            nc.scalar.dma_start(xbt_v[:, 4], xbt[4])
            nc.scalar.dma_start(wj_sb[:, 6:8, :], wjb[:, 2:4])

            psA = [psA_pool.tile([P, IB], F32, name="psA", tag="psA")
                   for _ in range(HT)]
            psB = [psB_pool.tile([P, L], F32, name="psB", tag="psB")
                   for _ in range(HT)]

            junk(3)

            # ---- bias rank-1 matmuls open each accumulation group ----
            for t in range(HT):
                nc.tensor.matmul(psA[t][:], brow_sb[:, t], ones_sb[:, :IB],
                                 start=True, stop=False)
            for t in range(HT):
                nc.tensor.matmul(psB[t][:], brow_sb[:, 2 + t], ones_sb[:],
                                 start=True, stop=False)

            # ---- layer 1: consume chunks in expected arrival order ----
            def a_chunks(ks, sp=False):
                for k in ks:
                    for t in range(HT):
                        nc.tensor.matmul(psA[t][:],
                                         wi_sb[:, k, t * P:(t + 1) * P],
                                         xbt_sb[:, k, :IB],
                                         start=False,
                                         stop=sp and k == ks[-1] and t == HT - 1)

            def b_chunks(ks, sp=False):
                for k in ks:
                    for t in range(HT):
                        nc.tensor.matmul(psB[t][:],
                                         wj_sb[:, k, t * P:(t + 1) * P],
                                         xbt_sb[:, k, :],
                                         start=False,
                                         stop=sp and k == ks[-1] and t == HT - 1)

            junk(4)
            a_chunks([0, 1])
            junk(2)
            a_chunks([2, 3])
            junk(2)
            b_chunks([0, 1])
            junk(1)
            a_chunks([4])
            b_chunks([2, 3])
            junk(1)
            a_chunks([5])
            a_chunks([6, 7])
            junk(1)
            a_chunks([8, 9], sp=True)
            junk(1)
            b_chunks([8, 9])
            junk(1)
            b_chunks([4, 5])
            # b(6,7) h-consume-split tail: t0 matmuls first so psB[0]
            # closes early and its relu ACTs overlap the t1 matmuls.
            for t in range(HT):
                for k in (6, 7):
                    nc.tensor.matmul(psB[t][:],
                                     wj_sb[:, k, t * P:(t + 1) * P],
                                     xbt_sb[:, k, :],
                                     start=False, stop=k == 7)
            junk(2)

            # ---- fused post-ops ----
            wo_b = lambda s, lo, hi: cst_sb[:, s, lo:hi, None].to_broadcast(
                (P, hi - lo, IB))
            psa_b = lambda lo, hi: psA[0][:, None, :].to_broadcast(
                (P, hi - lo, IB))

            def at0_op(sign, lo, hi):
                # fused t0: atp = max(psA,0)*Wo ; atm = min(psA,0)*(-Wo)
                dst = (atp_sb if sign == 0 else atm_sb)[:, 0, lo:hi]
                op0 = ALU.max if sign == 0 else ALU.min
                nc.vector.scalar_tensor_tensor(dst, psa_b(lo, hi), 0.0,
                                               wo_b(2 * sign, lo, hi),
                                               op0, ALU.mult)

            def at1_op(sign, lo, hi):
                # t1 on gpsimd from SBUF; both a-parts non-negative -> +Wo.
                src = ap1_sb if sign == 0 else am1_sb
                dst = (atp_sb if sign == 0 else atm_sb)[:, 1, lo:hi]
                nc.gpsimd.tensor_tensor(
                    dst, src[:, None, :].to_broadcast((P, hi - lo, IB)),
                    wo_b(1, lo, hi), ALU.mult)

            # vector: ap1 copy, fused t0 chain, then bm relus
            # at groups = the main loop's n-halves (0:5, 5:10)
            nc.vector.tensor_scalar_max(ap1_sb[:], psA[1][:], 0.0)
            at0_op(0, 0, 5)
            at0_op(1, 0, 5)
            nc.vector.tensor_scalar(bm_sb[:, 0], psB[0][:], -1.0, 0.0,
                                    ALU.mult, ALU.max)
            nc.vector.tensor_scalar(bm_sb[:, 1], psB[1][:], -1.0, 0.0,
                                    ALU.mult, ALU.max)
            at0_op(0, 5, 10)
            at0_op(1, 5, 10)
            # scalar: am1 copy, bp relus (psB[0] closes first)
            nc.scalar.activation(am1_sb[:], psA[1][:], RELU, scale=-1.0)
            nc.scalar.activation(bp_sb[:, 0], psB[0][:], RELU)
            nc.scalar.activation(bp_sb[:, 1], psB[1][:], RELU)
            # gpsimd: t1 chain
            at1_op(0, 0, 5)
            at1_op(1, 0, 5)
            at1_op(0, 5, 10)
            at1_op(1, 5, 10)

            # ---- main contraction, j-block major: stationary = b±t j-block
            # (M=128, vs M=96 with i-rows stationary), moving = at±
            # [n-half, i] (N=480).  25% fewer moving columns than the
            # i-major form.  Output bias enters as a rank-1 matmul; the
            # PSUM->fp16 copies alternate Scalar (ACT identity) / Vector.
            NH2 = NB // 2
            atp_v = atp_sb[:].rearrange("p t n i -> p t (n i)")
            atm_v = atm_sb[:].rearrange("p t n i -> p t (n i)")
            bor_v = bor_sb[:].rearrange("o n i -> o (n i)")
            for idx in range(6):
                nh, jb = idx // 3, idx % 3
                ns = slice(nh * NH2 * IB, (nh + 1) * NH2 * IB)
                js = slice(jb * P, (jb + 1) * P)
                psO = psO_pool.tile([P, NH2 * IB], F32, name="psO", tag="psO")
                nc.tensor.matmul(psO[:], ones_sb[:, :P], bor_v[:, ns],
                                 start=True, stop=False)
                nc.tensor.matmul(psO[:], bp_sb[:, 0, js], atp_v[:, 0, ns],
                                 start=False, stop=False)
                nc.tensor.matmul(psO[:], bm_sb[:, 0, js], atm_v[:, 0, ns],
                                 start=False, stop=False)
                nc.tensor.matmul(psO[:], bp_sb[:, 1, js], atp_v[:, 1, ns],
                                 start=False, stop=False)
                nc.tensor.matmul(psO[:], bm_sb[:, 1, js], atm_v[:, 1, ns],
                                 start=False, stop=True)
                ostage = stage_pool.tile([P, NH2, IB], F16, name="ostage",
                                         tag="ostage")
                ost_v = ostage[:].rearrange("p n i -> p (n i)")
                if idx % 2 == 0:
                    nc.scalar.activation(ost_v, psO[:], IDENT)
                    nc.scalar.dma_start(out[nh, jb], ostage[:])
                else:
                    nc.vector.tensor_copy(ost_v, psO[:])
                    nc.sync.dma_start(out[nh, jb], ostage[:])

    return nc


def _prep_inputs(x, Wi, bi, Wj, bj, Wo, bo):
    """Build the 8 per-core input maps."""
    f = lambda a: np.ascontiguousarray(np.asarray(a, dtype=np.float32))
    x, Wi, bi, Wj, bj, Wo, bo = map(f, (x, Wi, bi, Wj, bj, Wo, bo))

    # [1280, H] -> per-partition-contiguous [P, k-range, H] blocks
    def wpack(w, k0, k1):
        v = w.astype(np.float16).reshape(KT, P, H)[k0:k1]      # [k, P, H]
        return np.ascontiguousarray(v.transpose(1, 0, 2))      # [P, k, H]

    wi0_p, wi1_p = wpack(Wi, 0, 5), wpack(Wi, 5, 10)
    wja_p, wjb_p, wjc_p = wpack(Wj, 0, 4), wpack(Wj, 4, 8), wpack(Wj, 8, 10)

    wo_r = Wo.reshape(HT, P, NB).transpose(1, 0, 2)            # [128, 2, 10]
    cst = np.ascontiguousarray(np.stack(
        [wo_r[:, 0], wo_r[:, 1], -wo_r[:, 0], -wo_r[:, 1],
         np.tile(bo[None, :], (P, 1))], axis=1)).astype(np.float32)  # [128, 5, 10]
    brow = np.concatenate([bi.reshape(HT, P), bj.reshape(HT, P)],
                          axis=0)[None].astype(np.float16)     # [1, 4, 128]
    brow = np.ascontiguousarray(brow)
    bor = np.ascontiguousarray(
        np.tile(bo[:, None], (1, IB))[None].astype(np.float16))  # [1, 10, 96]

    xT = [x[b].T for b in range(B)]                            # [1280, 384]
    in_maps = []
    for c in range(NCORES):
        b, i0 = c // (NCORES // B), (c % (NCORES // B)) * IB
        xc = np.roll(xT[b], -i0, axis=1).astype(np.float16)
        xc = np.ascontiguousarray(xc.reshape(5, 2, P, L).transpose(0, 2, 1, 3))
        in_maps.append({"xbt": xc, "wi0": wi0_p, "wi1": wi1_p,
                        "wja": wja_p, "wjb": wjb_p, "wjc": wjc_p,
                        "cst": cst, "brow": brow, "bor": bor})
    return in_maps


def _run(inputs, trace=False):
    global _last_result
    nc = build_nc()
    if not nc.is_finalized():
        nc.finalize()
    in_maps = _prep_inputs(**inputs)
    res = bass_utils.run_bass_kernel_spmd(
        nc, in_maps, core_ids=list(range(NCORES)), trace=trace)
    _last_result = res
    full = np.empty((B, L, L, NB), dtype=np.float32)
    for c in range(NCORES):
        b, i0 = c // (NCORES // B), (c % (NCORES // B)) * IB
        o = res.results[c]["out"].astype(np.float32)   # [2, 3, 128, 5, 96]
        o = o.transpose(4, 1, 2, 0, 3).reshape(IB, L, NB)  # -> [i, j_rolled, n]
        full[b, i0:i0 + IB] = np.roll(o, i0, axis=1)
    return full


def kernel(**inputs):
    return _run(inputs, trace=False)


# revision 67
# speedup vs baseline: 1.1383x; 1.0101x over previous
"""
DistancePredictor Trainium2 kernel.

Math:
  xi = x @ Wi + bi            [B, L, H]
  xj = x @ Wj + bj            [B, L, H]
  out = relu(xi[:,:,None,:] * xj[:,None,:,:]) @ Wo + bo    [B, L, L, NB]

Key identity (exact, terms have disjoint support):
  relu(a*b) = relu(a)relu(b) + relu(-a)relu(-b)
so
  out[i,j,n] = sum_h (A+[i,h]B+[j,h] + A-[i,h]B-[j,h]) * Wo[h,n] + bo[n]
with A± = relu(±xi), B± = relu(±xj) — the whole pair/relu/contract
pipeline is pure TensorE matmuls; no [B,L,L,H] intermediate exists.

Signs as implemented:
  atp = max(psA,0) *  Wo      (psA = xi+bi, bias folded in via rank-1 matmul)
  atm = min(psA,0) * (-Wo)    (= relu(-(xi+bi)) * Wo)   [t0, fused from PSUM]
  atm = relu(-psA) *  Wo                                 [t1, via SBUF copy]
  bp  = max(psB,0)            (psB = xj+bj)
  bm  = max(-psB,0)           (= relu(-(xj+bj)))
  out[n] = atp·bp + atm·bm + bo[n]

Sharding: 8 cores; core c handles batch b=c//4 and i-rows
[96*(c%4), 96*(c%4)+96).  Weights replicated.

Schedule:
 - Inputs stream in packed layouts (1.5-2.5KB contiguous per partition
   per DMA) over both HW rings + one wi group via the gpsimd SWDGE
   path; the PE consumes chunks in arrival order (A data early, wj
   spread) so layer 1 finishes with the stream, and psA closes ~1us
   before the last wj so the at±-chain hides under the B tail.
 - Biases enter the PSUM accumulations as rank-1 matmuls (ones ⊗ b):
   post-accumulation ops are single fused DVE/ACT ops.
 - at±-chain split: Vector owns t0 (fused PSUM reads), GpSimd owns t1
   (from SBUF relu copies), Scalar owns bp + output biases.
 - Junk matmuls on a memset tile keep TensorE gapless so the HAM clock
   ramps to full (~5us of continuous activity) before the main loop.
 - Output staged per-n in fp16 (~5e-4 added rel err), drained on the
   sync ring during the main loop; the last n goes out on the scalar
   ring right behind its bias ACT.
"""

import numpy as np
import ml_dtypes

import concourse.bass as bass
import concourse.mybir as mybir
import concourse.tile as tile
from concourse import bacc, bass_utils

# Problem constants (hardcoded per contract).
B, L, D, H, NB = 2, 384, 1280, 256, 10
P = 128
KT = D // P     # 10 contraction chunks of 128
HT = H // P     # 2 h-chunks of 128
NCORES = 8
IB = (B * L) // NCORES   # 96 i-rows per core

F32 = mybir.dt.float32
F16 = mybir.dt.float16
ALU = mybir.AluOpType
RELU = mybir.ActivationFunctionType.Relu
IDENT = mybir.ActivationFunctionType.Identity

_last_result = None  # BassKernelResults of the most recent run (for test harness)


def build_nc():
    nc = bacc.Bacc("TRN2")

    xbt = nc.dram_tensor("xbt", [5, P, 2, L], F16, kind="ExternalInput")
    wi0 = nc.dram_tensor("wi0", [P, 5, H], F16, kind="ExternalInput")   # k0-4
    wi1 = nc.dram_tensor("wi1", [P, 5, H], F16, kind="ExternalInput")   # k5-9
    wja = nc.dram_tensor("wja", [P, 4, H], F16, kind="ExternalInput")   # k0-3
    wjb = nc.dram_tensor("wjb", [P, 4, H], F16, kind="ExternalInput")   # k4-7
    wjc = nc.dram_tensor("wjc", [P, 2, H], F16, kind="ExternalInput")   # k8-9
    # cst[:, 0:2] = Wo per h-chunk, [:, 2:4] = -Wo, [:, 4] = bo replicated
    cst = nc.dram_tensor("cst", [P, 5, NB], F32, kind="ExternalInput")
    # bias rows on one partition: [bi_t0, bi_t1, bj_t0, bj_t1]
    brow = nc.dram_tensor("brow", [1, 4, P], F16, kind="ExternalInput")
    # [n-half, j-block, j, n, i]: output in j-major blocks (M=128 matmuls)
    out = nc.dram_tensor("out", [2, 3, P, NB // 2, IB], F16, kind="ExternalOutput")

    with tile.TileContext(nc) as tc:
        with (
            tc.tile_pool(name="persist", bufs=1) as pp,
            tc.tile_pool(name="psA", bufs=2, space="PSUM") as psA_pool,
            tc.tile_pool(name="psB", bufs=2, space="PSUM") as psB_pool,
            tc.tile_pool(name="psO", bufs=4, space="PSUM") as psO_pool,
            tc.tile_pool(name="stage", bufs=4) as stage_pool,
        ):
            tl = lambda shape, name, dt=F32: pp.tile(shape, dt, name=name, tag=name)
            xbt_sb = tl([P, KT, L], "xbt_sb", F16)
            wi_sb = tl([P, KT, H], "wi_sb", F16)
            wj_sb = tl([P, KT, H], "wj_sb", F16)
            cst_sb = tl([P, 5, NB], "cst_sb")
            brow_sb = tl([1, 4, P], "brow_sb", F16)
            ones_sb = tl([1, L], "ones_sb", F16)

            bp_sb = tl([P, HT, L], "bp_sb", F16)         # relu(xj+bj)      [h, j]
            bm_sb = tl([P, HT, L], "bm_sb", F16)         # relu(-(xj+bj))
            atp_sb = tl([P, HT, NB, IB], "atp_sb", F16)  # [h, n, i]
            atm_sb = tl([P, HT, NB, IB], "atm_sb", F16)
            ap1_sb = tl([P, IB], "ap1_sb")               # max(psA1, 0)
            am1_sb = tl([P, IB], "am1_sb")               # relu(-psA1)

            warm_sb = tl([P, L], "warm_sb", F16)
            nc.vector.memset(warm_sb[:], 0.0)
            nc.vector.memset(ones_sb[:], 1.0)

            def junk(n_junk):
                # Full 128-partition matmuls: the HAM clock monitor only
                # counts wide-K PE streaming (K=32/64 never ramps).
                psW = psO_pool.tile([IB, L], F32, name="psW", tag="psO")
                for _ in range(n_junk):
                    nc.tensor.matmul(psW[:], warm_sb[:, :IB], warm_sb[:],
                                     start=True, stop=True,
                                     skip_group_check=True)

            # ---- DMA triggers.  Emission order per engine = issue order.
            # The rings share ~330-350GB/s of HBM; A-side data (xbt+wi)
            # is front-loaded on both rings so psA closes ~1us before the
            # stream ends, with wj groups spread in between so B matmuls
            # never bunch.  wjc is consumed last, h-split so psB[0]
            # closes before psB[1].
            xbt_v = xbt_sb[:].rearrange("p (g k) j -> p g k j", k=2)
            # jc rides the (otherwise idle) SWDGE path: ~130GB/s while the
            # HW rings are in their slow-start phase; takes 131KB off the
            # rings and lands the b(8,9) data by ~10.5us.
            nc.gpsimd.dma_start(wj_sb[:, 8:10, :], wjc[:])
            nc.gpsimd.dma_start(wj_sb[:, 2:4, :], wja[:, 2:4])
            nc.sync.dma_start(cst_sb[:], cst[:])
            nc.scalar.dma_start(brow_sb[:], brow[:])
            # sync: xbt0, xbt1, ja0(k0,1), xbt2, xbt3, jb0(k4,5), jc(k8,9)
            nc.sync.dma_start(xbt_v[:, 0], xbt[0])
            nc.sync.dma_start(xbt_v[:, 1], xbt[1])
            nc.sync.dma_start(wj_sb[:, 0:2, :], wja[:, 0:2])
            nc.sync.dma_start(xbt_v[:, 2], xbt[2])
            nc.sync.dma_start(xbt_v[:, 3], xbt[3])
            nc.sync.dma_start(wj_sb[:, 4:6, :], wjb[:, 0:2])
            # scalar: wi0, wi1, xbt4, jb1(k6,7)
            nc.scalar.dma_start(wi_sb[:, 0:5, :], wi0[:])
            nc.scalar.dma_start(wi_sb[:, 5:10, :], wi1[:])
            nc.scalar.dma_start(xbt_v[:, 4], xbt[4])
            nc.scalar.dma_start(wj_sb[:, 6:8, :], wjb[:, 2:4])

            psA = [psA_pool.tile([P, IB], F32, name="psA", tag="psA")
                   for _ in range(HT)]
            psB = [psB_pool.tile([P, L], F32, name="psB", tag="psB")
                   for _ in range(HT)]

            junk(3)

            # ---- bias rank-1 matmuls open each accumulation group ----
            for t in range(HT):
                nc.tensor.matmul(psA[t][:], brow_sb[:, t], ones_sb[:, :IB],
                                 start=True, stop=False)
            for t in range(HT):
                nc.tensor.matmul(psB[t][:], brow_sb[:, 2 + t], ones_sb[:],
                                 start=True, stop=False)

            # ---- layer 1: consume chunks in expected arrival order ----
            def a_chunks(ks, sp=False):
                for k in ks:
                    for t in range(HT):
                        nc.tensor.matmul(psA[t][:],
                                         wi_sb[:, k, t * P:(t + 1) * P],
                                         xbt_sb[:, k, :IB],
                                         start=False,
                                         stop=sp and k == ks[-1] and t == HT - 1)

            def b_chunks(ks, sp=False):
                for k in ks:
                    for t in range(HT):
                        nc.tensor.matmul(psB[t][:],
                                         wj_sb[:, k, t * P:(t + 1) * P],
                                         xbt_sb[:, k, :],
                                         start=False,
                                         stop=sp and k == ks[-1] and t == HT - 1)

            junk(4)
            a_chunks([0, 1])
            junk(2)
            a_chunks([2, 3])
            junk(2)
            b_chunks([0, 1])
            junk(1)
            a_chunks([4])
            b_chunks([2, 3])
            junk(1)
            a_chunks([5])
            a_chunks([6, 7])
            junk(1)
            a_chunks([8, 9], sp=True)
            junk(1)
            b_chunks([8, 9])
            junk(1)
            b_chunks([4, 5])
            # b(6,7) h-consume-split tail: t0 matmuls first so psB[0]
            # closes early and its relu ACTs overlap the t1 matmuls.
            for t in range(HT):
                for k in (6, 7):
                    nc.tensor.matmul(psB[t][:],
                                     wj_sb[:, k, t * P:(t + 1) * P],
                                     xbt_sb[:, k, :],
                                     start=False, stop=k == 7)
            junk(2)

            # ---- fused post-ops ----
            wo_b = lambda s, lo, hi: cst_sb[:, s, lo:hi, None].to_broadcast(
                (P, hi - lo, IB))
            psa_b = lambda lo, hi: psA[0][:, None, :].to_broadcast(
                (P, hi - lo, IB))

            def at0_op(sign, lo, hi):
                # fused t0: atp = max(psA,0)*Wo ; atm = min(psA,0)*(-Wo)
                dst = (atp_sb if sign == 0 else atm_sb)[:, 0, lo:hi]
                op0 = ALU.max if sign == 0 else ALU.min
                nc.vector.scalar_tensor_tensor(dst, psa_b(lo, hi), 0.0,
                                               wo_b(2 * sign, lo, hi),
                                               op0, ALU.mult)

            def at1_op(sign, lo, hi):
                # t1 on gpsimd from SBUF; both a-parts non-negative -> +Wo.
                src = ap1_sb if sign == 0 else am1_sb
                dst = (atp_sb if sign == 0 else atm_sb)[:, 1, lo:hi]
                nc.gpsimd.tensor_tensor(
                    dst, src[:, None, :].to_broadcast((P, hi - lo, IB)),
                    wo_b(1, lo, hi), ALU.mult)

            # vector: ap1 copy, fused t0 chain, then bm relus
            # at groups = the main loop's n-halves (0:5, 5:10)
            nc.vector.tensor_scalar_max(ap1_sb[:], psA[1][:], 0.0)
            at0_op(0, 0, 5)
            at0_op(1, 0, 5)
            nc.vector.tensor_scalar(bm_sb[:, 0], psB[0][:], -1.0, 0.0,
                                    ALU.mult, ALU.max)
            nc.vector.tensor_scalar(bm_sb[:, 1], psB[1][:], -1.0, 0.0,
                                    ALU.mult, ALU.max)
            at0_op(0, 5, 10)
            at0_op(1, 5, 10)
            # scalar: am1 copy, bp relus (psB[0] closes first)
            nc.scalar.activation(am1_sb[:], psA[1][:], RELU, scale=-1.0)
            nc.scalar.activation(bp_sb[:, 0], psB[0][:], RELU)
            nc.scalar.activation(bp_sb[:, 1], psB[1][:], RELU)
            # gpsimd: t1 chain
            at1_op(0, 0, 5)
            at1_op(1, 0, 5)
            at1_op(0, 5, 10)
            at1_op(1, 5, 10)

            # ---- main contraction, j-block major: stationary = b±t j-block
            # (M=128, vs M=96 with i-rows stationary), moving = at±
            # [n-half, i] (N=480).  25% fewer moving columns than the
            # i-major form.  Output bias enters as a rank-1 matmul; the
            # PSUM->fp16 copies alternate Scalar (ACT identity) / Vector.
            NH2 = NB // 2
            atp_v = atp_sb[:].rearrange("p t n i -> p t (n i)")
            atm_v = atm_sb[:].rearrange("p t n i -> p t (n i)")
            for idx in range(6):
                nh, jb = idx // 3, idx % 3
                ns = slice(nh * NH2 * IB, (nh + 1) * NH2 * IB)
                js = slice(jb * P, (jb + 1) * P)
                psO = psO_pool.tile([P, NH2 * IB], F32, name="psO", tag="psO")
                nc.tensor.matmul(psO[:], bp_sb[:, 0, js], atp_v[:, 0, ns],
                                 start=True, stop=False)
                nc.tensor.matmul(psO[:], bm_sb[:, 0, js], atm_v[:, 0, ns],
                                 start=False, stop=False)
                nc.tensor.matmul(psO[:], bp_sb[:, 1, js], atp_v[:, 1, ns],
                                 start=False, stop=False)
                nc.tensor.matmul(psO[:], bm_sb[:, 1, js], atm_v[:, 1, ns],
                                 start=False, stop=True)
                # bias rides the fp16 convert as a broadcast add on Vector
                # (cst row 4 = bo replicated across partitions); no PE time.
                ostage = stage_pool.tile([P, NH2, IB], F16, name="ostage",
                                         tag="ostage")
                psO_3 = psO[:].rearrange("p (n i) -> p n i", i=IB)
                bo_b = cst_sb[:, 4, nh * NH2:(nh + 1) * NH2, None].to_broadcast(
                    (P, NH2, IB))
                nc.vector.tensor_tensor(ostage[:], psO_3, bo_b, ALU.add)
                eng = nc.scalar if idx % 2 == 0 else nc.sync
                eng.dma_start(out[nh, jb], ostage[:])

    return nc


def _prep_inputs(x, Wi, bi, Wj, bj, Wo, bo):
    """Build the 8 per-core input maps."""
    f = lambda a: np.ascontiguousarray(np.asarray(a, dtype=np.float32))
    x, Wi, bi, Wj, bj, Wo, bo = map(f, (x, Wi, bi, Wj, bj, Wo, bo))

    # [1280, H] -> per-partition-contiguous [P, k-range, H] blocks
    def wpack(w, k0, k1):
        v = w.astype(np.float16).reshape(KT, P, H)[k0:k1]      # [k, P, H]
        return np.ascontiguousarray(v.transpose(1, 0, 2))      # [P, k, H]

    wi0_p, wi1_p = wpack(Wi, 0, 5), wpack(Wi, 5, 10)
    wja_p, wjb_p, wjc_p = wpack(Wj, 0, 4), wpack(Wj, 4, 8), wpack(Wj, 8, 10)

    wo_r = Wo.reshape(HT, P, NB).transpose(1, 0, 2)            # [128, 2, 10]
    cst = np.ascontiguousarray(np.stack(
        [wo_r[:, 0], wo_r[:, 1], -wo_r[:, 0], -wo_r[:, 1],
         np.tile(bo[None, :], (P, 1))], axis=1)).astype(np.float32)  # [128, 5, 10]
    brow = np.concatenate([bi.reshape(HT, P), bj.reshape(HT, P)],
                          axis=0)[None].astype(np.float16)     # [1, 4, 128]
    brow = np.ascontiguousarray(brow)

    xT = [x[b].T for b in range(B)]                            # [1280, 384]
    in_maps = []
    for c in range(NCORES):
        b, i0 = c // (NCORES // B), (c % (NCORES // B)) * IB
        xc = np.roll(xT[b], -i0, axis=1).astype(np.float16)
        xc = np.ascontiguousarray(xc.reshape(5, 2, P, L).transpose(0, 2, 1, 3))
        in_maps.append({"xbt": xc, "wi0": wi0_p, "wi1": wi1_p,
                        "wja": wja_p, "wjb": wjb_p, "wjc": wjc_p,
                        "cst": cst, "brow": brow})
    return in_maps


def _run(inputs, trace=False):
    global _last_result
    nc = build_nc()
    if not nc.is_finalized():
        nc.finalize()
    in_maps = _prep_inputs(**inputs)
    res = bass_utils.run_bass_kernel_spmd(
        nc, in_maps, core_ids=list(range(NCORES)), trace=trace)
    _last_result = res
    full = np.empty((B, L, L, NB), dtype=np.float32)
    for c in range(NCORES):
        b, i0 = c // (NCORES // B), (c % (NCORES // B)) * IB
        o = res.results[c]["out"].astype(np.float32)   # [2, 3, 128, 5, 96]
        o = o.transpose(4, 1, 2, 0, 3).reshape(IB, L, NB)  # -> [i, j_rolled, n]
        full[b, i0:i0 + IB] = np.roll(o, i0, axis=1)
    return full


def kernel(**inputs):
    return _run(inputs, trace=False)


# revision 68
# speedup vs baseline: 1.1672x; 1.0255x over previous
"""
DistancePredictor Trainium2 kernel.

Math:
  xi = x @ Wi + bi            [B, L, H]
  xj = x @ Wj + bj            [B, L, H]
  out = relu(xi[:,:,None,:] * xj[:,None,:,:]) @ Wo + bo    [B, L, L, NB]

Key identity (exact, terms have disjoint support):
  relu(a*b) = relu(a)relu(b) + relu(-a)relu(-b)
so
  out[i,j,n] = sum_h (A+[i,h]B+[j,h] + A-[i,h]B-[j,h]) * Wo[h,n] + bo[n]
with A± = relu(±xi), B± = relu(±xj) — the whole pair/relu/contract
pipeline is pure TensorE matmuls; no [B,L,L,H] intermediate exists.

Signs as implemented:
  atp = max(psA,0) *  Wo      (psA = xi+bi, bias folded in via rank-1 matmul)
  atm = min(psA,0) * (-Wo)    (= relu(-(xi+bi)) * Wo)   [t0, fused from PSUM]
  atm = relu(-psA) *  Wo                                 [t1, via SBUF copy]
  bp  = max(psB,0)            (psB = xj+bj)
  bm  = max(-psB,0)           (= relu(-(xj+bj)))
  out[n] = atp·bp + atm·bm + bo[n]

Sharding: 8 cores; core c handles batch b=c//4 and i-rows
[96*(c%4), 96*(c%4)+96).  Weights replicated.

Schedule:
 - Inputs stream in packed layouts (1.5-2.5KB contiguous per partition
   per DMA) over both HW rings + one wi group via the gpsimd SWDGE
   path; the PE consumes chunks in arrival order (A data early, wj
   spread) so layer 1 finishes with the stream, and psA closes ~1us
   before the last wj so the at±-chain hides under the B tail.
 - Biases enter the PSUM accumulations as rank-1 matmuls (ones ⊗ b):
   post-accumulation ops are single fused DVE/ACT ops.
 - at±-chain split: Vector owns t0 (fused PSUM reads), GpSimd owns t1
   (from SBUF relu copies), Scalar owns am1/bp relus.
 - Junk matmuls on a memset tile keep TensorE gapless so the HAM clock
   ramps to full (~5us of continuous activity) before the main loop.
 - Main contraction is j-block major: stationary = b±t j-block (M=128,
   full PE width), moving = at± [n-half, i] (N=480) — 25% fewer moving
   columns than the i-major form (11520 vs 15360).
 - bo + fp32->fp16 conversion fuse into one Vector broadcast-add per
   output block (~5e-4 added rel err from fp16 out); blocks drain on
   alternating rings during the main loop.
"""

import numpy as np
import ml_dtypes

import concourse.bass as bass
import concourse.mybir as mybir
import concourse.tile as tile
from concourse import bacc, bass_utils

# Problem constants (hardcoded per contract).
B, L, D, H, NB = 2, 384, 1280, 256, 10
P = 128
KT = D // P     # 10 contraction chunks of 128
HT = H // P     # 2 h-chunks of 128
NCORES = 8
IB = (B * L) // NCORES   # 96 i-rows per core

F32 = mybir.dt.float32
F16 = mybir.dt.float16
ALU = mybir.AluOpType
RELU = mybir.ActivationFunctionType.Relu
IDENT = mybir.ActivationFunctionType.Identity

_last_result = None  # BassKernelResults of the most recent run (for test harness)


def build_nc():
    nc = bacc.Bacc("TRN2")

    xbt = nc.dram_tensor("xbt", [5, P, 2, L], F16, kind="ExternalInput")
    wi0 = nc.dram_tensor("wi0", [P, 5, H], F16, kind="ExternalInput")   # k0-4
    wi1 = nc.dram_tensor("wi1", [P, 5, H], F16, kind="ExternalInput")   # k5-9
    wja = nc.dram_tensor("wja", [P, 4, H], F16, kind="ExternalInput")   # k0-3
    wjb = nc.dram_tensor("wjb", [P, 4, H], F16, kind="ExternalInput")   # k4-7
    wjc = nc.dram_tensor("wjc", [P, 2, H], F16, kind="ExternalInput")   # k8-9
    # cst[:, 0:2] = Wo per h-chunk, [:, 2:4] = -Wo, [:, 4] = bo replicated
    cst = nc.dram_tensor("cst", [P, 5, NB], F32, kind="ExternalInput")
    # bias rows on one partition: [bi_t0, bi_t1, bj_t0, bj_t1]
    brow = nc.dram_tensor("brow", [1, 4, P], F16, kind="ExternalInput")
    # [n-half, j-block, j, n, i]: output in j-major blocks (M=128 matmuls)
    out = nc.dram_tensor("out", [2, 3, P, NB // 2, IB], F16, kind="ExternalOutput")

    with tile.TileContext(nc) as tc:
        with (
            tc.tile_pool(name="persist", bufs=1) as pp,
            tc.tile_pool(name="psA", bufs=2, space="PSUM") as psA_pool,
            tc.tile_pool(name="psB", bufs=2, space="PSUM") as psB_pool,
            tc.tile_pool(name="psO", bufs=4, space="PSUM") as psO_pool,
            tc.tile_pool(name="stage", bufs=4) as stage_pool,
        ):
            tl = lambda shape, name, dt=F32: pp.tile(shape, dt, name=name, tag=name)
            xbt_sb = tl([P, KT, L], "xbt_sb", F16)
            wi_sb = tl([P, KT, H], "wi_sb", F16)
            wj_sb = tl([P, KT, H], "wj_sb", F16)
            cst_sb = tl([P, 5, NB], "cst_sb")
            brow_sb = tl([1, 4, P], "brow_sb", F16)
            ones_sb = tl([1, L], "ones_sb", F16)

            bp_sb = tl([P, HT, L], "bp_sb", F16)         # relu(xj+bj)      [h, j]
            bm_sb = tl([P, HT, L], "bm_sb", F16)         # relu(-(xj+bj))
            atp_sb = tl([P, HT, NB, IB], "atp_sb", F16)  # [h, n, i]
            atm_sb = tl([P, HT, NB, IB], "atm_sb", F16)
            ap1_sb = tl([P, IB], "ap1_sb")               # max(psA1, 0)
            am1_sb = tl([P, IB], "am1_sb")               # relu(-psA1)

            warm_sb = tl([P, L], "warm_sb", F16)
            nc.vector.memset(warm_sb[:], 0.0)
            nc.vector.memset(ones_sb[:], 1.0)

            def junk(n_junk):
                # Full 128-partition matmuls: the HAM clock monitor only
                # counts wide-K PE streaming (K=32/64 never ramps).
                psW = psO_pool.tile([IB, L], F32, name="psW", tag="psO")
                for _ in range(n_junk):
                    nc.tensor.matmul(psW[:], warm_sb[:, :IB], warm_sb[:],
                                     start=True, stop=True,
                                     skip_group_check=True)

            # ---- DMA triggers.  Emission order per engine = issue order.
            # The rings share ~330-350GB/s of HBM; A-side data (xbt+wi)
            # is front-loaded on both rings so psA closes ~1us before the
            # stream ends, with wj groups spread in between so B matmuls
            # never bunch.  wjc is consumed last, h-split so psB[0]
            # closes before psB[1].
            xbt_v = xbt_sb[:].rearrange("p (g k) j -> p g k j", k=2)
            # jc rides the (otherwise idle) SWDGE path: ~130GB/s while the
            # HW rings are in their slow-start phase; takes 131KB off the
            # rings and lands the b(8,9) data by ~10.5us.
            nc.gpsimd.dma_start(wj_sb[:, 8:10, :], wjc[:])
            nc.gpsimd.dma_start(wj_sb[:, 2:4, :], wja[:, 2:4])
            nc.sync.dma_start(cst_sb[:], cst[:])
            nc.scalar.dma_start(brow_sb[:], brow[:])
            # sync: xbt0, xbt1, ja0(k0,1), xbt2, xbt3, jb0(k4,5), jc(k8,9)
            nc.sync.dma_start(xbt_v[:, 0], xbt[0])
            nc.sync.dma_start(xbt_v[:, 1], xbt[1])
            nc.sync.dma_start(wj_sb[:, 0:2, :], wja[:, 0:2])
            nc.sync.dma_start(xbt_v[:, 2], xbt[2])
            nc.sync.dma_start(xbt_v[:, 3], xbt[3])
            nc.sync.dma_start(wj_sb[:, 4:6, :], wjb[:, 0:2])
            # scalar: wi0, wi1, xbt4, jb1(k6,7)
            nc.scalar.dma_start(wi_sb[:, 0:5, :], wi0[:])
            nc.scalar.dma_start(wi_sb[:, 5:10, :], wi1[:])
            nc.scalar.dma_start(xbt_v[:, 4], xbt[4])
            nc.scalar.dma_start(wj_sb[:, 6:8, :], wjb[:, 2:4])

            psA = [psA_pool.tile([P, IB], F32, name="psA", tag="psA")
                   for _ in range(HT)]
            psB = [psB_pool.tile([P, L], F32, name="psB", tag="psB")
                   for _ in range(HT)]

            junk(3)

            # ---- bias rank-1 matmuls open each accumulation group ----
            for t in range(HT):
                nc.tensor.matmul(psA[t][:], brow_sb[:, t], ones_sb[:, :IB],
                                 start=True, stop=False)
            for t in range(HT):
                nc.tensor.matmul(psB[t][:], brow_sb[:, 2 + t], ones_sb[:],
                                 start=True, stop=False)

            # ---- layer 1: consume chunks in expected arrival order ----
            def a_chunks(ks, sp=False):
                for k in ks:
                    for t in range(HT):
                        nc.tensor.matmul(psA[t][:],
                                         wi_sb[:, k, t * P:(t + 1) * P],
                                         xbt_sb[:, k, :IB],
                                         start=False,
                                         stop=sp and k == ks[-1] and t == HT - 1)

            def b_chunks(ks, sp=False):
                for k in ks:
                    for t in range(HT):
                        nc.tensor.matmul(psB[t][:],
                                         wj_sb[:, k, t * P:(t + 1) * P],
                                         xbt_sb[:, k, :],
                                         start=False,
                                         stop=sp and k == ks[-1] and t == HT - 1)

            junk(4)
            a_chunks([0, 1])
            junk(2)
            a_chunks([2, 3])
            junk(2)
            b_chunks([0, 1])
            junk(1)
            a_chunks([4])
            b_chunks([2, 3])
            junk(1)
            a_chunks([5])
            a_chunks([6, 7])
            junk(1)
            a_chunks([8, 9], sp=True)
            junk(1)
            b_chunks([8, 9])
            junk(1)
            b_chunks([4, 5])
            # b(6,7) h-consume-split tail: t0 matmuls first so psB[0]
            # closes early and its relu ACTs overlap the t1 matmuls.
            for t in range(HT):
                for k in (6, 7):
                    nc.tensor.matmul(psB[t][:],
                                     wj_sb[:, k, t * P:(t + 1) * P],
                                     xbt_sb[:, k, :],
                                     start=False, stop=k == 7)
            junk(2)

            # ---- fused post-ops ----
            wo_b = lambda s, lo, hi: cst_sb[:, s, lo:hi, None].to_broadcast(
                (P, hi - lo, IB))
            psa_b = lambda lo, hi: psA[0][:, None, :].to_broadcast(
                (P, hi - lo, IB))

            def at0_op(sign, lo, hi):
                # fused t0: atp = max(psA,0)*Wo ; atm = min(psA,0)*(-Wo)
                dst = (atp_sb if sign == 0 else atm_sb)[:, 0, lo:hi]
                op0 = ALU.max if sign == 0 else ALU.min
                nc.vector.scalar_tensor_tensor(dst, psa_b(lo, hi), 0.0,
                                               wo_b(2 * sign, lo, hi),
                                               op0, ALU.mult)

            def at1_op(sign, lo, hi):
                # t1 on gpsimd from SBUF; both a-parts non-negative -> +Wo.
                src = ap1_sb if sign == 0 else am1_sb
                dst = (atp_sb if sign == 0 else atm_sb)[:, 1, lo:hi]
                nc.gpsimd.tensor_tensor(
                    dst, src[:, None, :].to_broadcast((P, hi - lo, IB)),
                    wo_b(1, lo, hi), ALU.mult)

            # vector: ap1 copy, fused t0 chain, then bm relus
            # at groups = the main loop's n-halves (0:5, 5:10)
            nc.vector.tensor_scalar_max(ap1_sb[:], psA[1][:], 0.0)
            at0_op(0, 0, 5)
            at0_op(1, 0, 5)
            nc.vector.tensor_scalar(bm_sb[:, 0], psB[0][:], -1.0, 0.0,
                                    ALU.mult, ALU.max)
            nc.vector.tensor_scalar(bm_sb[:, 1], psB[1][:], -1.0, 0.0,
                                    ALU.mult, ALU.max)
            at0_op(0, 5, 10)
            at0_op(1, 5, 10)
            # scalar: am1 copy, bp relus (psB[0] closes first)
            nc.scalar.activation(am1_sb[:], psA[1][:], RELU, scale=-1.0)
            nc.scalar.activation(bp_sb[:, 0], psB[0][:], RELU)
            nc.scalar.activation(bp_sb[:, 1], psB[1][:], RELU)
            # gpsimd: t1 chain
            at1_op(0, 0, 5)
            at1_op(1, 0, 5)
            at1_op(0, 5, 10)
            at1_op(1, 5, 10)

            # ---- main contraction, j-block major: stationary = b±t j-block
            # (M=128, vs M=96 with i-rows stationary), moving = at±
            # [n-half, i] (N=480).  25% fewer moving columns than the
            # i-major form.  Output bias enters as a rank-1 matmul; the
            # PSUM->fp16 copies alternate Scalar (ACT identity) / Vector.
            NH2 = NB // 2
            atp_v = atp_sb[:].rearrange("p t n i -> p t (n i)")
            atm_v = atm_sb[:].rearrange("p t n i -> p t (n i)")
            for idx in range(6):
                nh, jb = idx // 3, idx % 3
                ns = slice(nh * NH2 * IB, (nh + 1) * NH2 * IB)
                js = slice(jb * P, (jb + 1) * P)
                psO = psO_pool.tile([P, NH2 * IB], F32, name="psO", tag="psO")
                nc.tensor.matmul(psO[:], bp_sb[:, 0, js], atp_v[:, 0, ns],
                                 start=True, stop=False)
                nc.tensor.matmul(psO[:], bm_sb[:, 0, js], atm_v[:, 0, ns],
                                 start=False, stop=False)
                nc.tensor.matmul(psO[:], bp_sb[:, 1, js], atp_v[:, 1, ns],
                                 start=False, stop=False)
                nc.tensor.matmul(psO[:], bm_sb[:, 1, js], atm_v[:, 1, ns],
                                 start=False, stop=True)
                # bias rides the fp16 convert as a broadcast add on Vector
                # (cst row 4 = bo replicated across partitions); no PE time.
                ostage = stage_pool.tile([P, NH2, IB], F16, name="ostage",
                                         tag="ostage")
                psO_3 = psO[:].rearrange("p (n i) -> p n i", i=IB)
                bo_b = cst_sb[:, 4, nh * NH2:(nh + 1) * NH2, None].to_broadcast(
                    (P, NH2, IB))
                nc.vector.tensor_tensor(ostage[:], psO_3, bo_b, ALU.add)
                eng = nc.scalar if idx % 2 == 0 else nc.sync
                eng.dma_start(out[nh, jb], ostage[:])

    return nc


def _prep_inputs(x, Wi, bi, Wj, bj, Wo, bo):
    """Build the 8 per-core input maps."""
    f = lambda a: np.ascontiguousarray(np.asarray(a, dtype=np.float32))
    x, Wi, bi, Wj, bj, Wo, bo = map(f, (x, Wi, bi, Wj, bj, Wo, bo))

    # [1280, H] -> per-partition-contiguous [P, k-range, H] blocks
    def wpack(w, k0, k1):
        v = w.astype(np.float16).reshape(KT, P, H)[k0:k1]      # [k, P, H]
        return np.ascontiguousarray(v.transpose(1, 0, 2))      # [P, k, H]

    wi0_p, wi1_p = wpack(Wi, 0, 5), wpack(Wi, 5, 10)
    wja_p, wjb_p, wjc_p = wpack(Wj, 0, 4), wpack(Wj, 4, 8), wpack(Wj, 8, 10)

    wo_r = Wo.reshape(HT, P, NB).transpose(1, 0, 2)            # [128, 2, 10]
    cst = np.ascontiguousarray(np.stack(
        [wo_r[:, 0], wo_r[:, 1], -wo_r[:, 0], -wo_r[:, 1],
         np.tile(bo[None, :], (P, 1))], axis=1)).astype(np.float32)  # [128, 5, 10]
    brow = np.concatenate([bi.reshape(HT, P), bj.reshape(HT, P)],
                          axis=0)[None].astype(np.float16)     # [1, 4, 128]
    brow = np.ascontiguousarray(brow)

    xT = [x[b].T for b in range(B)]                            # [1280, 384]
    in_maps = []
    for c in range(NCORES):
        b, i0 = c // (NCORES // B), (c % (NCORES // B)) * IB
        xc = np.roll(xT[b], -i0, axis=1).astype(np.float16)
        xc = np.ascontiguousarray(xc.reshape(5, 2, P, L).transpose(0, 2, 1, 3))
        in_maps.append({"xbt": xc, "wi0": wi0_p, "wi1": wi1_p,
                        "wja": wja_p, "wjb": wjb_p, "wjc": wjc_p,
                        "cst": cst, "brow": brow})
    return in_maps


def _run(inputs, trace=False):
    global _last_result
    nc = build_nc()
    if not nc.is_finalized():
        nc.finalize()
    in_maps = _prep_inputs(**inputs)
    res = bass_utils.run_bass_kernel_spmd(
        nc, in_maps, core_ids=list(range(NCORES)), trace=trace)
    _last_result = res
    full = np.empty((B, L, L, NB), dtype=np.float32)
    for c in range(NCORES):
        b, i0 = c // (NCORES // B), (c % (NCORES // B)) * IB
        o = res.results[c]["out"].astype(np.float32)   # [2, 3, 128, 5, 96]
        o = o.transpose(4, 1, 2, 0, 3).reshape(IB, L, NB)  # -> [i, j_rolled, n]
        full[b, i0:i0 + IB] = np.roll(o, i0, axis=1)
    return full


def kernel(**inputs):
    return _run(inputs, trace=False)
